# revision 1
# baseline (speedup 1.0000x reference)
"""Trainium2 Bass kernel for BinaryMaskEdgeSmoothing.

Reference computation (per image, SAME-padded 3x3 convs):
    e  = conv3x3(x, lap)
    em = sigmoid(|e| * 3)
    b  = conv3x3(x, gauss)
    smoothed = x*(1-em) + b*em
    out = (smoothed > 0.5).astype(f32)

Device decomposition (per NeuronCore, data-parallel over B*C=64 images,
8 images per core):

  * Images are processed in row-tiles of 128 rows (partition dim = image
    rows, free dim = the 1024 columns).  The vertical direction of each
    3x3 conv is computed on the TensorEngine as a banded-matrix matmul
    (lhsT[p, m] = k[dy, dx] at p = m+dy-1); the horizontal taps are
    free-dim shifts of the moving operand, accumulated into the same
    PSUM bank over the three dx matmuls.  So each conv costs 3 matmuls
    per 512-column PSUM bank and the full 3x3 conv lands in PSUM with
    zero vector-engine work.
  * Rather than computing gauss and then b-x, the PE directly computes
    d = conv(x, gauss - delta) = b - x  (delta = unit impulse), which is
    exact and saves elementwise work.
  * The moving operand is the *negated* mask in bf16 (exact for a 0/1
    mask).  conv(-x) = -conv(x); |e| is unaffected and the sign of d is
    absorbed into the final fused compare:
        out = 1  iff  em*(b-x) + x > 0.5  iff  t_neg + 0.5 < x
    where t_neg = em * conv(-x, gauss-delta).  That final compare is a
    single fused GpSimd scalar_tensor_tensor op.
  * Row tiles overlap by 2 rows (stride 126) so every output row has its
    vertical halo inside one tile; rows 1009..1023 of all 8 images are
    batched into one final tile with block-diagonal band matrices.

Engine budget per [128,1024] unit: PE 12 matmuls, ACT abs+sigmoid,
DVE cast+mul, GPSIMD fused compare, DMA 0.5MiB in + ~0.5MiB out.
Everything lands near the per-core HBM roofline (~64MiB @ ~360GB/s).
"""

import numpy as np
import ml_dtypes

import concourse.bass as bass
import concourse.bacc as bacc
import concourse.mybir as mybir
import concourse.tile as tile
from concourse.bass_utils import run_bass_kernel_spmd

Af = mybir.ActivationFunctionType
Op = mybir.AluOpType
F32 = mybir.dt.float32
BF16 = mybir.dt.bfloat16

N_CORES = 8
B_PER_CORE = 8
H = 1024
W = 1024

# Row tiling: main units at i0 = 126*k, each 128 input rows.
#   k == 0 : output rows 0..126   (partitions 0..126)
#   k >= 1 : output rows i0+1..i0+126 (partitions 1..126)
# Tail: remaining rows handled by one batched unit across all images.


def _tiling(h):
    n_main = (h - 128) // 126 + 1
    covered_max = 126 * (n_main - 1) + 126
    r = h - 1 - covered_max  # rows left for the tail unit
    return n_main, r


def build_weights(lap_kernel, gauss_kernel, b_imgs=B_PER_CORE, h=H):
    """Host-side: 12 [128,128] banded lhsT matrices -> [128, 12*128] bf16.

    idx = kern*3 + dx        : mid-tile band (shared by all main units)
    idx = 6 + kern*3 + dx    : tail-unit block-diagonal band
    """
    lap = np.asarray(lap_kernel, dtype=np.float64).reshape(3, 3)
    gau = np.asarray(gauss_kernel, dtype=np.float64).reshape(3, 3)
    gmd = gau.copy()
    gmd[1, 1] -= 1.0  # gauss - delta  ->  conv = b - x
    kerns = [lap, gmd]

    _, r = _tiling(h)
    s = r + 1  # tail block height (input rows per image in tail unit)

    mats = np.zeros((12, 128, 128), dtype=np.float64)
    for ki, k3 in enumerate(kerns):
        for dx in range(3):
            m = mats[ki * 3 + dx]
            for out_m in range(128):
                for dy in range(3):
                    p = out_m + dy - 1
                    if 0 <= p < 128:
                        m[p, out_m] = k3[dy, dx]
            mt = mats[6 + ki * 3 + dx]
            if r > 0:
                for blk in range(b_imgs):
                    base = blk * s
                    for out_m in range(s):
                        for dy in range(3):
                            p = out_m + dy - 1
                            if 0 <= p < s:
                                mt[base + p, base + out_m] = k3[dy, dx]
    # stack as [p, idx, m] so one DMA drops it straight into SBUF
    w = np.transpose(mats, (1, 0, 2)).reshape(128, 12 * 128)
    return np.ascontiguousarray(w.astype(np.float32)).astype(ml_dtypes.bfloat16)


def build_nc(b_imgs=B_PER_CORE, h=H, w=W, rep=1, rep_loop=0, ablate=None, store_engine="scalar", load_engine="sync", lg8=False, xin_bufs=3):
    n_main, r = _tiling(h)
    s = r + 1
    if r > 0:
        assert b_imgs * s <= 128, (b_imgs, s)
    # W chunking into PSUM banks (<=512 f32 per matmul free dim)
    chunks = []
    c0 = 0
    while c0 < w:
        chunks.append((c0, min(c0 + 512, w)))
        c0 += 512

    nc = bacc.Bacc()
    x_d = nc.declare_dram_parameter("x", [b_imgs, h, w], F32, isOutput=False)
    w_d = nc.declare_dram_parameter("wts", [128, 12 * 128], BF16, isOutput=False)
    o_d = nc.declare_dram_parameter("out", [b_imgs, h, w], F32, isOutput=True)

    with tile.TileContext(nc) as tc:
        with (
            tc.tile_pool(name="const", bufs=1) as cpool,
            tc.tile_pool(name="xin", bufs=xin_bufs) as xpool,
            tc.tile_pool(name="work", bufs=4) as wpool,
            tc.tile_pool(name="oput", bufs=3) as opool,
            tc.tile_pool(name="psum", bufs=4, space="PSUM") as ppool,
        ):
            st_eng = nc.scalar if store_engine == "scalar" else nc.sync
            ld_eng = nc.scalar if load_engine == "scalar" else nc.sync
            wsb = cpool.tile([128, 12 * 128], BF16)
            nc.sync.dma_start(wsb[:], w_d[:])
            neghalf = cpool.tile([128, w], F32)
            nc.vector.memset(neghalf[:], -0.5)

            def lhsT(variant, kern, dx, kpart=128):
                idx = variant * 6 + kern * 3 + dx
                return wsb[0:kpart, idx * 128:(idx + 1) * 128]

            def lhsT_m(variant, kern, dx, kpart, mpart):
                idx = variant * 6 + kern * 3 + dx
                return wsb[0:kpart, idx * 128:idx * 128 + mpart]

            def conv_unit(xf, xneg, variant, o_ap, kpart=128, mpart=128):
                """emit 12 matmuls + pointwise chain; writes o_ap.

                One PSUM bank per (conv, 512-col chunk): e/d tags get
                bufs=4 half-width slots, so the PE can run up to 4 chunk
                convs ahead of the pointwise consumers."""
                for (a, b) in chunks:
                    cw = b - a
                    e_ps = ppool.tile([mpart, cw], F32, tag="e")
                    d_ps = ppool.tile([mpart, cw], F32, tag="d")
                    for kern, ps in ((0, e_ps), (1, d_ps)):
                        # center tap (dx=1) covers the whole bank: start=True
                        nc.tensor.matmul(
                            ps[:], lhsT_m(variant, kern, 1, kpart, mpart),
                            xneg[:, a:b], start=True, stop=False)
                        # left neighbor (dx=0): out[:, j] += k*x[:, j-1]
                        la = max(a, 1)
                        nc.tensor.matmul(
                            ps[:, la - a:cw],
                            lhsT_m(variant, kern, 0, kpart, mpart),
                            xneg[:, la - 1:b - 1], start=False, stop=False)
                        # right neighbor (dx=2): out[:, j] += k*x[:, j+1]
                        rb = min(b, w - 1)
                        nc.tensor.matmul(
                            ps[:, 0:rb - a],
                            lhsT_m(variant, kern, 2, kpart, mpart),
                            xneg[:, a + 1:rb + 1], start=False, stop=True)
                    a_sb = wpool.tile([mpart, cw], F32, tag="a")
                    nc.scalar.activation(a_sb[:], e_ps[:], Af.Abs)
                    if ablate == "conv":
                        nc.scalar.activation(o_ap[:, a:b], d_ps[:], Af.Abs)
                        continue
                    em_sb = wpool.tile([mpart, cw], F32, tag="em")
                    nc.scalar.activation(
                        em_sb[:], a_sb[:], Af.Sigmoid, scale=3.0)
                    t_sb = wpool.tile([mpart, cw], F32, tag="t")
                    nc.vector.tensor_tensor(
                        t_sb[:], em_sb[:], d_ps[:], Op.mult)
                    # out = ((t_neg + 0.5) < x)  <=>  smoothed > 0.5
                    # fused on DVE; GPSIMD unused (its TT ucode is ~2x
                    # slower and blocks on the shared SBUF port)
                    nc.vector.scalar_tensor_tensor(
                        o_ap[:, a:b], t_sb[:], 0.5, xf[:, a:b],
                        Op.add, Op.is_lt)

            import contextlib

            def body_ctx():
                if rep_loop:
                    return tc.For_i(0, rep_loop, 1)
                return contextlib.nullcontext()

            # LG row-tiles per load transfer, SG per store transfer
            # (loads on the SP HWDGE ring, stores separate so store waits
            # never stall the load FIFO)
            LG = 8 if (lg8 and n_main % 8 == 0) else (4 if n_main % 4 == 0 else 1)
            SG = min(4, LG)

            with body_ctx():
              for _ in range(rep):
                for b in range(b_imgs):
                    for l0 in range(0, n_main, LG):
                        lg = min(LG, n_main - l0)
                        xt = xpool.tile([128, lg, w], F32, tag="xf")
                        ld_eng.dma_start(
                            xt[:],
                            bass.AP(x_d, (b * h + 126 * l0) * w,
                                    [[w, 128], [126 * w, lg], [1, w]]))
                        if ablate != "dma":
                            xneg = xpool.tile([128, lg, w], BF16, tag="xn")
                            nc.vector.tensor_scalar(
                                xneg[:], xt[:], -1.0, None, Op.mult)
                        for k0 in range(l0, l0 + lg, SG):
                            gu = min(SG, l0 + lg - k0)
                            if ablate == "dma":
                                o_grp = xt
                                osl = slice(k0 - l0, k0 - l0 + gu)
                            else:
                                o_grp = opool.tile(
                                    [128, gu, w], F32, tag="o")
                                osl = slice(0, gu)
                                for j in range(gu):
                                    u = k0 - l0 + j
                                    conv_unit(xt[:, u, :], xneg[:, u, :],
                                              0, o_grp[:, j, :])
                            # batched store: rows 126*k0+1 ..
                            st_eng.dma_start(
                                bass.AP(o_d, (b * h + 126 * k0 + 1) * w,
                                        [[w, 126], [126 * w, gu], [1, w]]),
                                o_grp[1:127, osl, :])
                            if k0 == 0:
                                st_eng.dma_start(
                                    o_d[b, 0:1, :], o_grp[0:1, osl.start, :])

                if r > 0:
                    kpart = b_imgs * s
                    hc = h - s
                    xft = xpool.tile([kpart, w], F32, tag="xft")
                    ld_eng.dma_start(xft[:], x_d[:, hc:h, :])
                    xnt = xpool.tile([kpart, w], BF16, tag="xnt")
                    nc.vector.tensor_scalar(
                        xnt[:], xft[:], -1.0, None, Op.mult)
                    o_t = opool.tile([kpart, w], F32, tag="ot")
                    conv_unit(xft[:], xnt[:], 1, o_t[:], kpart, kpart)
                    for b in range(b_imgs):
                        st_eng.dma_start(
                            o_d[b, hc + 1:h, :], o_t[b * s + 1:(b + 1) * s, :])

    return nc


_NC_CACHE = {}


def _get_nc(key=(B_PER_CORE, H, W)):
    if key not in _NC_CACHE:
        nc = build_nc(*key)
        nc.finalize()
        _NC_CACHE[key] = nc
    return _NC_CACHE[key]


def kernel(mask, lap_kernel, gauss_kernel):
    mask = np.ascontiguousarray(np.asarray(mask, dtype=np.float32))
    bb, cc, h, w = mask.shape
    assert (h, w) == (H, W) and bb * cc == N_CORES * B_PER_CORE
    x_all = mask.reshape(N_CORES * B_PER_CORE, h, w)
    wts = build_weights(lap_kernel, gauss_kernel)

    nc = _get_nc()
    in_maps = [
        {"x": np.ascontiguousarray(x_all[c * B_PER_CORE:(c + 1) * B_PER_CORE]),
         "wts": wts}
        for c in range(N_CORES)
    ]
    res = run_bass_kernel_spmd(nc, in_maps, list(range(N_CORES)))
    out = np.stack([res.results[c]["out"] for c in range(N_CORES)])
    return out.reshape(bb, cc, h, w).astype(np.float32)



# revision 5
# speedup vs baseline: 3.2387x; 3.2387x over previous
"""Trainium2 Bass kernel for BinaryMaskEdgeSmoothing.

Reference computation (per image, SAME-padded 3x3 convs):
    e  = conv3x3(x, lap)
    em = sigmoid(|e| * 3)
    b  = conv3x3(x, gauss)
    smoothed = x*(1-em) + b*em
    out = (smoothed > 0.5).astype(f32)

Key insight: the mask is binary, so every conv output is determined by
the 3x3 neighborhood pattern.  box = conv(x, ones3x3) and g16 =
16*conv(x, gauss) are small integers, and exhaustively checking all 512
neighborhood patterns against the f32 reference shows the whole
pipeline collapses to ONE linear threshold:

    out = [ conv3x3(x, K) > 10.125 ],
    K   = 16*gauss - 0.25*lap + 3.5*delta
        = [[1.25, 2.25, 1.25],
           [2.25, 5.50, 2.25],
           [1.25, 2.25, 1.25]]

(z = g16 + box/4 + 5x/4 uniquely separates the reference's decision
boundary, including its f32 sigmoid saturation behavior at |e|>=6 and
all zero-padded borders.)  All K values and x in {0,1} are exact in
fp8e4, products are exact, and PSUM accumulates in f32, so the kernel
is bit-exact vs the reference.

Device decomposition (per NeuronCore, data-parallel over B*C=64 images,
8 images per core):

  * Row-tiles of 128 input rows (partition dim), stride 126; the
    vertical direction of the conv is a banded-matrix matmul
    (lhsT[p, m] = K[dy, dx] at p = m+dy-1); horizontal taps are free-dim
    shifts of the moving operand.  Band truncation at partitions 0/127
    implements the zero padding at image top/bottom edges.
  * x is loaded as fp8 (host-cast, exact for a 0/1 mask) into a
    width-padded SBUF tile with zeroed guard columns, so horizontal
    shifts never need edge-special matmuls.
  * fp8 DoubleRow perf mode contracts 2 k-tiles per pass at 0.5
    cycles/output-row: the (dx=0, dx=2) taps pair into one matmul
    (their K columns are equal), and (dx=1, zero-band) forms the other.
    A 3x3 conv costs just 2 matmul instructions per 512-col PSUM chunk.
  * The threshold compare is a single DVE tensor_scalar is_gt reading
    PSUM f32 and writing the fp8 {0,1} output tile; output is stored as
    fp8 (8 MiB/core) and widened to f32 on the host.
  * Rows 1009..1023 of all 8 images batch into one final 128-partition
    tile with block-diagonal band matrices.

Engine budget per core: DMA ~16.2 MiB (~47 us at 358 GB/s, the
bottleneck), PE 260 DoubleRow matmuls (~28 us), DVE 130 compares
(~35 us).  Everything else idle.
"""

import numpy as np
import ml_dtypes

import concourse.bass as bass
import concourse.bacc as bacc
import concourse.mybir as mybir
import concourse.tile as tile
from concourse.bass_utils import run_bass_kernel_spmd

Op = mybir.AluOpType
F32 = mybir.dt.float32
FP8 = mybir.dt.float8e4
NP_FP8 = mybir.dt.np(FP8)  # ml_dtypes.float8_e4m3

N_CORES = 8
B_PER_CORE = 8
H = 1024
W = 1024

WPAD = 1028  # padded SBUF row: x col j lives at c = 2 + j; c in {1,1026} zero
XOFF = 2
NMAIN = 8    # main row-tiles per image: in rows [126k, 126k+128), k=0..7
TAIL_S = 16  # tail block: in rows 1008..1023 of each image (out 1009..1023)
THRESH = 10.125


def _edge_kernel(lap_kernel, gauss_kernel):
    lap = np.asarray(lap_kernel, dtype=np.float64).reshape(3, 3)
    gau = np.asarray(gauss_kernel, dtype=np.float64).reshape(3, 3)
    K = 16.0 * gau - 0.25 * lap
    K[1, 1] += 3.5
    return K


def build_weights(lap_kernel, gauss_kernel):
    """Host-side: 8 banded [128,128] lhsT matrices -> [128, 8*128] fp8.

    Index i = variant*4 + pair*2 + t:
      variant 0: main band (p = m+dy-1), variant 1: tail block-diagonal
      pair 0: t=0 -> K col 0, t=1 -> K col 2   (dx=0 / dx=2 taps)
      pair 1: t=0 -> K col 1, t=1 -> zeros     (dx=1 tap)
    """
    K = _edge_kernel(lap_kernel, gauss_kernel)
    cols = {(0, 0): 0, (0, 1): 2, (1, 0): 1, (1, 1): None}
    mats = np.zeros((2, 2, 2, 128, 128), dtype=np.float64)
    for pair in range(2):
        for t in range(2):
            col = cols[(pair, t)]
            if col is None:
                continue
            m_main = mats[0, pair, t]
            for m in range(128):
                for dy in range(3):
                    p = m + dy - 1
                    if 0 <= p < 128:
                        m_main[p, m] = K[dy, col]
            m_tail = mats[1, pair, t]
            for blk in range(B_PER_CORE):
                base = blk * TAIL_S
                for ml in range(1, TAIL_S):
                    for dy in range(3):
                        p = ml + dy - 1
                        if 0 <= p < TAIL_S:
                            m_tail[base + p, base + ml] = K[dy, col]
    # stack as [p, i, m] so one DMA drops it straight into SBUF
    w = np.transpose(mats.reshape(8, 128, 128), (1, 0, 2)).reshape(128, 8 * 128)
    return np.ascontiguousarray(w.astype(np.float32)).astype(NP_FP8)


def build_nc(b_imgs=B_PER_CORE, h=H, w=W):
    # main units cover out rows 0..1008; tail covers 1009..1023
    assert 126 * (NMAIN - 1) + 126 + TAIL_S == h
    chunks = [(0, 512), (512, 1024)]
    PM = mybir.MatmulPerfMode.DoubleRow

    nc = bacc.Bacc()
    x_d = nc.declare_dram_parameter("x", [b_imgs, h, w], FP8, isOutput=False)
    w_d = nc.declare_dram_parameter("wts", [128, 8 * 128], FP8, isOutput=False)
    o_d = nc.declare_dram_parameter("out", [b_imgs, h, w], FP8, isOutput=True)

    with tile.TileContext(nc) as tc:
        with (
            tc.tile_pool(name="const", bufs=1) as cpool,
            tc.tile_pool(name="xin", bufs=3) as xpool,
            tc.tile_pool(name="oput", bufs=3) as opool,
            tc.tile_pool(name="psum", bufs=4, space="PSUM") as ppool,
        ):
            wsb = cpool.tile([128, 8, 128], FP8)
            nc.sync.dma_start(wsb[:], w_d[:])

            def xrhs(xt, u, c0):
                """[128, 2, 512] moving AP: reads cols c0+2t+j of unit u."""
                ap = xt[:]
                pstride = ap.ap[0][0]
                return bass.AP(ap.tensor, u * WPAD + c0,
                               [[pstride, 128], [2, 2], [1, 512]])

            def conv_unit(xt, u, variant, o_ap):
                """One [128, w] unit: 4 matmuls + 2 compares -> o_ap."""
                for (a, b) in chunks:
                    ps = ppool.tile([128, b - a], F32, tag="ps")
                    nc.tensor.matmul(
                        ps[:], wsb[:, variant * 4 + 0:variant * 4 + 2, :],
                        xrhs(xt, u, 1 + a), start=True, stop=False,
                        perf_mode=PM)
                    nc.tensor.matmul(
                        ps[:], wsb[:, variant * 4 + 2:variant * 4 + 4, :],
                        xrhs(xt, u, 2 + a), start=False, stop=True,
                        perf_mode=PM)
                    nc.vector.tensor_scalar(
                        o_ap[:, a:b], ps[:], THRESH, None, Op.is_gt)

            SG = 4  # units per store group
            for b in range(b_imgs):
                xt = xpool.tile([128, NMAIN, WPAD], FP8, tag="xf")
                nc.sync.dma_start(
                    xt[:, :, XOFF:XOFF + w],
                    bass.AP(x_d, b * h * w, [[w, 128], [126 * w, NMAIN], [1, w]]))
                nc.gpsimd.memset(xt[:, :, 0:XOFF], 0.0)
                nc.gpsimd.memset(xt[:, :, XOFF + w:WPAD], 0.0)
                for k0 in range(0, NMAIN, SG):
                    o_grp = opool.tile([128, SG, w], FP8, tag="o")
                    for j in range(SG):
                        conv_unit(xt, k0 + j, 0, o_grp[:, j, :])
                    nc.scalar.dma_start(
                        bass.AP(o_d, (b * h + 126 * k0 + 1) * w,
                                [[w, 126], [126 * w, SG], [1, w]]),
                        o_grp[1:127, :, :])
                    if k0 == 0:
                        nc.scalar.dma_start(
                            o_d[b, 0:1, :], o_grp[0:1, 0, :])

            # tail: rows 1008..1023 of all images, block-diagonal bands
            xtt = xpool.tile([128, 1, WPAD], FP8, tag="xt")
            nc.sync.dma_start(
                xtt[:, 0, XOFF:XOFF + w], x_d[:, h - TAIL_S:h, :])
            nc.gpsimd.memset(xtt[:, :, 0:XOFF], 0.0)
            nc.gpsimd.memset(xtt[:, :, XOFF + w:WPAD], 0.0)
            o_t = opool.tile([128, w], FP8, tag="ot")
            conv_unit(xtt, 0, 1, o_t[:])
            for b in range(b_imgs):
                nc.scalar.dma_start(
                    o_d[b, h - TAIL_S + 1:h, :],
                    o_t[b * TAIL_S + 1:(b + 1) * TAIL_S, :])

    return nc


_NC_CACHE = {}


def _get_nc(key=(B_PER_CORE, H, W)):
    if key not in _NC_CACHE:
        nc = build_nc(*key)
        nc.finalize()
        _NC_CACHE[key] = nc
    return _NC_CACHE[key]


def make_in_maps(mask, lap_kernel, gauss_kernel):
    mask = np.asarray(mask)
    bb, cc, h, w = mask.shape
    assert (h, w) == (H, W) and bb * cc == N_CORES * B_PER_CORE
    x_all = np.ascontiguousarray(mask.reshape(N_CORES * B_PER_CORE, h, w))
    x_fp8 = x_all.astype(NP_FP8)
    wts = build_weights(lap_kernel, gauss_kernel)
    return [
        {"x": np.ascontiguousarray(x_fp8[c * B_PER_CORE:(c + 1) * B_PER_CORE]),
         "wts": wts}
        for c in range(N_CORES)
    ]


def kernel(mask, lap_kernel, gauss_kernel):
    mask = np.asarray(mask, dtype=np.float32)
    bb, cc, h, w = mask.shape
    in_maps = make_in_maps(mask, lap_kernel, gauss_kernel)
    nc = _get_nc()
    res = run_bass_kernel_spmd(nc, in_maps, list(range(N_CORES)))
    out = np.stack([res.results[c]["out"] for c in range(N_CORES)])
    return out.reshape(bb, cc, h, w).astype(np.float32)
